# revision 1
# baseline (speedup 1.0000x reference)
"""Trainium2 kernel for nn_EuclideanEmbedding (edge-scale + segment_sum), v6.

Computes: out[n, :] = inv * sum_{e: receivers[e]==n} sh_vectors[e, :] * cutoffs[e]

Distribution: edges sharded across the 8 NeuronCores by receiver node range
(core c owns nodes [c*6250, (c+1)*6250)); each core emits its disjoint slice
of the output, so no collective is needed.

Device pipeline per core (fp16 data, f32 PSUM accumulate):
  1. DMA.  Measured HWDGE behavior under 8-core load: per-line cost is
     ~225ns + bytes/27GB/s per SDMA engine, so throughput is set by the
     per-partition LINE LENGTH (7.36MB moved in 32us with 5-16KB lines but
     22us with 60KB lines).  Everything (sh slots + cutoffs) therefore
     lives in ONE [128, W] host rectangle -- super-groups side by side,
     junk rows zeroed -- split into ~5 unequal column chunks (big first,
     tiny last to keep the multiply tail short), alternating between the
     two HWDGE rings (Sync + Scalar/ACT).
  2. scl = sh * cut in place (2x DVE mode: node columns innermost step-1;
     the d-broadcast of cut rides a 0-stride middle AP dim).  One SG
     (~15% of volume) runs on GpSimd; its chunk lands first (slow engine
     needs lead time) but its windows are processed LAST so the Vector
     pipeline is never blocked behind it.
  3. PE: segmented slot-sum as one matmul per 512-column window with a
     block-ones stationary (inv folded in on-device by one ScalarE op).
     out[m, (d,ng)] = sum_s scl[(m*c+s), (d,ng)] accumulates in PSUM f32.
     Four windows pack one PSUM bank at partition offsets {0,32,64,96}
     (tile_position col-tiling).
  4. ScalarE: one PSUM->SBUF fp16 copy per bank evicts 4 windows at once
     into staging tiles.
  5. Two chunked staging->HBM DMAs (DMA cannot read PSUM).

Windows: nodes are degree-sorted; a window is npack*32 consecutive ranks
with slot capacity c = roundup(max deg, 4), npack = 128//c.  Same-c windows
merge into super-groups sharing one multiply.
"""

import os

import numpy as np

# ---------------------------------------------------------------- constants
N_NODES = 50_000
D_SH = 16
N_CORES = 8
NPC = N_NODES // N_CORES          # 6250 nodes per core
NPAD = 6400                       # degree-rank space per core (>= NPC)
NG = 32                           # node columns per window (16*NG = 512 = max N)
CAP_Q = 4                         # slot-capacity quantum
NCOL = D_SH * NG                  # 512 moving columns per window matmul
SGW = NCOL + NG                   # fused sh+cut columns per window
CLW = 4                           # windows per PSUM bank / evict cluster
GP_FRAC = 0.16                    # target share of multiply FD for GpSimd
CHUNK_MAX = 2_600_000             # max bytes per input DMA chunk pair

_NC_CACHE: dict = {}
LAST_RESULTS = None  # BassKernelResults of the most recent run (for test.py)


# ---------------------------------------------------------------- planning
def plan_windows(D):
    """Rank-ordered windows/super-groups from the cross-core max degree
    profile D (sorted descending, len NPAD)."""
    q0 = 0
    raw = []
    while q0 < NPAD:
        d0 = int(D[q0]) if q0 < len(D) else 0
        c = max(CAP_Q, -(-d0 // CAP_Q) * CAP_Q)
        c = min(c, 128)
        npk = min(128 // c, 32)
        raw.append((q0, c, npk))
        q0 += npk * NG
    sgs = []
    rank_wins = []  # (q0, c, npk, sg_idx, v)
    for (w0, c, npk) in raw:
        if sgs and sgs[-1][0] == c and sgs[-1][1] == npk:
            sgs[-1][2] += 1
        else:
            sgs.append([c, npk, 1])
        rank_wins.append((w0, c, npk, len(sgs) - 1, sgs[-1][2] - 1))
    return tuple(tuple(s) for s in sgs), rank_wins


def device_plan(sgs):
    """Deterministic device plan derived from the SG tuple.

    chunks: list of (p, [si...]) per input DMA -- same-p SGs share one
      tightly-packed [p, W] rectangle (gp SG alone, first; then big to
      small so the last chunk has a short multiply tail)
    win_sgs: SG indices in window (matmul) order (gp last)
    gbase / gwidth / coloff: flat sh-buffer layout per chunk / SG
    m_base: ones-stationary column base per SG
    """
    n_sg = len(sgs)
    fd = [s[2] * NCOL for s in sgs]
    tot = sum(fd)
    gp_si = -1
    best = None
    for si in range(n_sg):
        if fd[si] <= 0.25 * tot:
            d = abs(fd[si] - GP_FRAC * tot)
            if best is None or d < best:
                best, gp_si = d, si
    p_of = [sgs[si][1] * sgs[si][0] for si in range(n_sg)]
    sgbytes = [p_of[si] * sgs[si][2] * SGW * 2 for si in range(n_sg)]
    vsis = sorted((si for si in range(n_sg) if si != gp_si),
                  key=lambda si: -sgbytes[si])
    vchunks, cur, cb = [], [], 0
    for si in vsis:
        if cur and cb + sgbytes[si] > CHUNK_MAX:
            vchunks.append((max(p_of[s] for s in cur), cur))
            cur, cb = [], 0
        cur.append(si)
        cb += sgbytes[si]
    if cur:
        vchunks.append((max(p_of[s] for s in cur), cur))
    chunks = ([(p_of[gp_si], [gp_si])] if gp_si >= 0 else []) + vchunks
    win_sgs = [si for (p, sis) in vchunks for si in sis]
    if gp_si >= 0:
        win_sgs.append(gp_si)
    gbase, gwidth, coloff = [0], [], [0] * n_sg
    for (p, sis) in chunks:
        off = 0
        for si in sis:
            coloff[si] = off
            off += sgs[si][2] * SGW
        gwidth.append(off)
        gbase.append(gbase[-1] + p * off)
    m_base = [0] * (n_sg + 1)
    for i in range(n_sg):
        m_base[i + 1] = m_base[i] + sgs[i][1]
    return {"gp_si": gp_si, "chunks": chunks, "win_sgs": win_sgs,
            "gbase": gbase, "gwidth": gwidth, "coloff": coloff,
            "m_base": m_base}


def dev_windows(sgs, plan):
    out = []
    for si in plan["win_sgs"]:
        c, npk, nw = sgs[si]
        for v in range(nw):
            out.append((si, v, c, npk))
    return out


# ---------------------------------------------------------------- device IR
def build_nc(sgs):
    key = tuple(sgs)
    if key in _NC_CACHE:
        return _NC_CACHE[key]

    import concourse.bacc as bacc
    import concourse.bass as bass
    import concourse.mybir as mybir
    from concourse import tile

    plan = device_plan(sgs)
    m_base, coloff = plan["m_base"], plan["coloff"]
    windows = dev_windows(sgs, plan)
    n_win = len(windows)
    n_cl = -(-n_win // CLW)
    OUTW = n_cl * NCOL
    maxnpk = max(s[1] for s in sgs)
    hstage = 32 * (CLW - 1) + maxnpk   # rows actually carrying data
    # ~3 clusters per output chunk: transfers overlap the input stream
    ocl = max(1, -(-n_cl // 5))
    ochunks = [(a, min(a + ocl, n_cl)) for a in range(0, n_cl, ocl)]

    nc = bacc.Bacc("TRN2", target_bir_lowering=False, debug=False)
    f16 = mybir.dt.float16
    f32 = mybir.dt.float32

    sh = nc.dram_tensor("sh", [plan["gbase"][-1]], f16,
                        kind="ExternalInput")
    ones = nc.dram_tensor("ones", [128, m_base[-1]], f16, kind="ExternalInput")
    inv = nc.dram_tensor("inv", [128, 1], f32, kind="ExternalInput")
    out = nc.dram_tensor("out", [hstage, OUTW], f16, kind="ExternalOutput")

    with tile.TileContext(nc) as tc:
        with (
            tc.tile_pool(name="data", bufs=1) as dpool,
            tc.psum_pool(name="ps", bufs=8) as pspool,
        ):
            inv_t = dpool.tile([128, 1], f32)
            nc.scalar.dma_start(inv_t[:], inv[:])
            ones_t = dpool.tile([128, m_base[-1]], f16)
            nc.scalar.dma_start(ones_t[:], ones[:])

            # input chunks alternate rings; a chunk tile holds its SGs'
            # fused sh+cut columns, tightly packed at partition height p
            ch_t = {}
            for ki, (p, sis) in enumerate(plan["chunks"]):
                w = plan["gwidth"][ki]
                t = dpool.tile([p, w], f16, tag=f"ch{ki}", name=f"ch{ki}")
                eng = nc.sync if ki % 2 == 0 else nc.scalar
                eng.dma_start(t[:], bass.AP(sh.ap().tensor,
                                            int(plan["gbase"][ki]),
                                            [[w, p], [1, w]]))
                for si in sis:
                    ch_t[si] = (t, coloff[si])

            # fold inv into the block-ones stationary (ScalarE, one op)
            ones2 = dpool.tile([128, m_base[-1]], f16)
            nc.scalar.activation(ones2[:], ones_t[:],
                                 mybir.ActivationFunctionType.Copy,
                                 bias=0.0, scale=inv_t[:])

            # scl = sh * cut in place; gp SG split into ~3 ops
            def emit_mul(eng, si, v0, nv):
                c, npk, nw = sgs[si]
                p = npk * c
                t, o = ch_t[si]
                a = t[:p, o + v0 * NCOL:o + (v0 + nv) * NCOL]
                b = t[:p, o + nw * NCOL + v0 * NG:
                      o + nw * NCOL + (v0 + nv) * NG]
                sh4 = bass.AP(a.tensor, a.offset,
                              [list(a.ap[0]), [NCOL, nv], [NG, D_SH],
                               [1, NG]])
                cut4 = bass.AP(b.tensor, b.offset,
                               [list(b.ap[0]), [NG, nv], [0, D_SH], [1, NG]])
                eng.tensor_mul(sh4, sh4, cut4)

            gp_si = plan["gp_si"]
            if gp_si >= 0:
                nwg = sgs[gp_si][2]
                step = max(1, -(-nwg // 3))
                for v0 in range(0, nwg, step):
                    emit_mul(nc.gpsimd, gp_si, v0, min(step, nwg - v0))
            for si in plan["win_sgs"]:
                if si != gp_si:
                    emit_mul(nc.vector, si, 0, sgs[si][2])

            stage = []
            for ci, (a, b) in enumerate(ochunks):
                stage.append(dpool.tile([hstage, (b - a) * NCOL], f16,
                                        tag=f"st{ci}", name=f"st{ci}"))

            ps_t = None
            for wi, (si, v, c, npk) in enumerate(windows):
                j = wi % CLW
                cl = wi // CLW
                if j == 0:
                    ps_t = pspool.tile([128, NCOL], f32, tag="ps",
                                       name=f"ps{cl}")
                p = npk * c
                t, o = ch_t[si]
                rhs = t[:p, o + v * NCOL:o + (v + 1) * NCOL]
                lhsT = ones2[:p, m_base[si]:m_base[si] + npk]
                nc.tensor.matmul(ps_t[32 * j:32 * j + npk, :], lhsT, rhs,
                                 start=True, stop=True,
                                 tile_position=(0, 32 * j))
                if j == CLW - 1 or wi == n_win - 1:
                    ci = next(i for i, (a, b) in enumerate(ochunks)
                              if a <= cl < b)
                    a, b = ochunks[ci]
                    nc.scalar.activation(
                        stage[ci][:, (cl - a) * NCOL:(cl - a + 1) * NCOL],
                        ps_t[:hstage, :],
                        mybir.ActivationFunctionType.Copy)
                    if cl == b - 1:
                        nc.sync.dma_start(
                            bass.AP(out.ap().tensor, a * NCOL,
                                    [[OUTW, hstage],
                                     [1, (b - a) * NCOL]]),
                            stage[ci][:])

    nc.compile()
    _NC_CACHE[key] = nc
    return nc


# ---------------------------------------------------------------- host shard
def shard_inputs(sh_vectors, cutoffs, receivers, inv_avg_num_neighbors):
    sh_np = np.ascontiguousarray(np.asarray(sh_vectors, dtype=np.float32))
    cut_np = np.asarray(cutoffs, dtype=np.float32).ravel()
    rec = np.asarray(receivers).astype(np.int64).ravel()
    inv_val = np.float32(np.asarray(inv_avg_num_neighbors).ravel()[0])

    order = np.argsort(rec, kind="stable")
    rec_sorted = rec[order]
    first = np.searchsorted(rec_sorted, rec_sorted, side="left")
    occ = np.arange(rec.size) - first            # occurrence within node
    bounds = np.searchsorted(rec_sorted, np.arange(0, N_NODES + 1, NPC))

    degs = np.zeros((N_CORES, NPAD), dtype=np.int64)
    node_orders, pos_of_node = [], []
    for c in range(N_CORES):
        lseg = rec_sorted[bounds[c]:bounds[c + 1]] - c * NPC
        d = np.bincount(lseg, minlength=NPAD)
        degs[c] = d
        no = np.argsort(-d, kind="stable")       # rank q -> local node id
        node_orders.append(no)
        pon = np.empty(NPAD, dtype=np.int64)
        pon[no] = np.arange(NPAD)
        pos_of_node.append(pon)

    D = np.sort(degs, axis=1)[:, ::-1].max(axis=0)   # cross-core max profile
    sgs, rank_wins = plan_windows(D)
    plan = device_plan(sgs)
    m_base, coloff = plan["m_base"], plan["coloff"]
    n_sg = len(sgs)
    chunk_of_sg = [0] * n_sg
    for ki, (p, sis) in enumerate(plan["chunks"]):
        for si in sis:
            chunk_of_sg[si] = ki

    q0_arr = np.array([w[0] for w in rank_wins], dtype=np.int64)
    c_of_w = np.array([w[1] for w in rank_wins], dtype=np.int64)
    sg_of_w = np.array([w[3] for w in rank_wins], dtype=np.int64)
    v_of_w = np.array([w[4] for w in rank_wins], dtype=np.int64)
    gb_of_sg = np.array([plan["gbase"][chunk_of_sg[si]]
                         for si in range(n_sg)], dtype=np.int64)
    gw_of_sg = np.array([plan["gwidth"][chunk_of_sg[si]]
                         for si in range(n_sg)], dtype=np.int64)
    co_of_sg = np.array([coloff[si] for si in range(n_sg)], dtype=np.int64)
    cutco_of_sg = np.array([coloff[si] + sgs[si][2] * NCOL
                            for si in range(n_sg)], dtype=np.int64)

    ones_dev = np.zeros((128, m_base[-1]), dtype=np.float16)
    for si, (c, npk, nw) in enumerate(sgs):
        for m in range(npk):
            ones_dev[m * c:(m + 1) * c, m_base[si] + m] = 1.0
    inv_dev = np.full((128, 1), inv_val, dtype=np.float32)

    in_maps = []
    for core in range(N_CORES):
        lo, hi = bounds[core], bounds[core + 1]
        edges = order[lo:hi]
        l = rec_sorted[lo:hi] - core * NPC
        o = occ[lo:hi]
        q = pos_of_node[core][l]
        w = np.searchsorted(q0_arr, q, side="right") - 1
        dq = q - q0_arr[w]
        m = dq // NG
        ng = dq - m * NG
        ce = c_of_w[w]
        sgi = sg_of_w[w]
        row = m * ce + o
        base = gb_of_sg[sgi] + row * gw_of_sg[sgi]
        shflat = base + co_of_sg[sgi] + v_of_w[w] * NCOL + ng
        cutflat = base + cutco_of_sg[sgi] + v_of_w[w] * NG + ng

        sh_dev = np.zeros(plan["gbase"][-1], dtype=np.float16)
        shv = sh_np[edges].astype(np.float16)
        for d in range(D_SH):
            sh_dev[shflat + d * NG] = shv[:, d]
        sh_dev[cutflat] = cut_np[edges].astype(np.float16)
        in_maps.append({"sh": sh_dev, "ones": ones_dev, "inv": inv_dev})
    return in_maps, sgs, rank_wins, node_orders


# ---------------------------------------------------------------- profiling
def _install_ntff_shim() -> bool:
    try:
        import sys
        import types

        import antenv

        if getattr(antenv, "axon_hooks", None) is not None:
            return True
        import trn_agent_boot.trn_boot as tb

        hook = tb._ntff_profile_via_ctypes("/opt/axon/libaxon_pjrt.so")
        mod = types.ModuleType("antenv.axon_hooks")
        mod._hook = hook
        mod.get_axon_ntff_profile_hook = lambda: mod._hook
        mod.set_axon_ntff_profile_hook = lambda h: setattr(mod, "_hook", h)
        sys.modules["antenv.axon_hooks"] = mod
        antenv.axon_hooks = mod
        return hook is not None
    except Exception as e:  # profiling is best-effort; the run must not break
        print(f"ntff shim unavailable: {e!r}")
        return False


# ---------------------------------------------------------------- entrypoint
def kernel(sh_vectors, cutoffs, receivers, inv_avg_num_neighbors) -> np.ndarray:
    global LAST_RESULTS
    from concourse.bass_utils import run_bass_kernel_spmd

    in_maps, sgs, rank_wins, node_orders = shard_inputs(
        sh_vectors, cutoffs, receivers, inv_avg_num_neighbors)
    nc = build_nc(sgs)

    trace = os.environ.get("KERNEL_TRACE", "0") == "1"
    if trace:
        trace = _install_ntff_shim()
    res = run_bass_kernel_spmd(nc, in_maps, core_ids=list(range(N_CORES)),
                               trace=trace)
    LAST_RESULTS = res

    plan = device_plan(sgs)
    windows = dev_windows(sgs, plan)
    devpos = {}
    for wi, (si, v, c, npk) in enumerate(windows):
        devpos[(si, v)] = wi

    npadd = rank_wins[-1][0] + rank_wins[-1][2] * NG
    full = np.empty((N_NODES, D_SH), dtype=np.float32)
    for core in range(N_CORES):
        r = res.results[core]["out"].astype(np.float32)  # [hstage, ...]
        res_rank = np.empty((npadd, D_SH), dtype=np.float32)
        for (q0, c, npk, si, v) in rank_wins:
            wi = devpos[(si, v)]
            cl, j = wi // CLW, wi % CLW
            blk = r[32 * j:32 * j + npk,
                    cl * NCOL:(cl + 1) * NCOL].reshape(npk, D_SH, NG)
            res_rank[q0:q0 + npk * NG] = blk.transpose(0, 2, 1).reshape(
                npk * NG, D_SH)
        blk_full = np.empty((NPAD, D_SH), dtype=np.float32)
        blk_full[node_orders[core]] = res_rank[:NPAD]
        full[core * NPC:(core + 1) * NPC] = blk_full[:NPC]
    return full

